# revision 23
# baseline (speedup 1.0000x reference)
"""Multi-head self-attention (CMHSAttn) Trainium2 kernel.

Problem: x (1, 128, 64, 64) fp32, W_qkv (384, 128) fp32.
  qkv = 1x1-conv(x, W_qkv); per head h (8 heads, d_head=16):
  q,k,v from qkv channels [48h:48h+16], [48h+16:48h+32], [48h+32:48h+48];
  out = softmax(q k^T / sqrt(128)) v, laid out channel-major (128, 64, 64).

Sharding: one head per NeuronCore (8 cores), pure data parallel, no
collectives. Each core receives the full x (bf16, channel-major (128, 4096))
plus its head's weight slices, and computes its 16 output channels.

Per-core algorithm (all matmuls bf16 with fp32 PSUM accumulation):
  - QT/KT = W_{q,k} @ x, computed replicated at partition offsets 0/32/64
    so score matmuls can be packed 3-wide into PE row groups (K=16 only).
  - V2 (128, 48*32): per 128-position chunk kj, cols [48kj:48kj+16] = V chunk
    (position-major), [+16:+32] zeros, [+32:+48] = 1.0 (softmax-denominator
    rows; the zero padding keeps partition bases 32-aligned downstream).
  - For each q-chunk (512) and k-group (3 k-tiles of 128): S^T tiles
    (k-partition, q-free) via packed matmuls -> one ACT exp over (128, 1536)
    PSUM -> P bf16 -> matmuls accumulate O'' (48, 512) += V2_kj^T @ P_kj.
    O'' rows 0:16 = unnormalized output^T, rows 32:48 = softmax denominator.
  - out^T = O''[0:16] * reciprocal(O''[32:48]); DMA to HBM.

Optimization notes from the DVE-offload experiments (2026-08-08): splitting
the exp between ACT and a DVE Schraudolph bit-trick (int16(s*c1+c2) bitcast
to bf16; rel err ~6e-3 end-to-end, validated on HW) is numerically safe but
measured SLOWER (152-186us vs 147.7us): each DVE op pays a pipe-drain about
equal to its own duration, and the in-order PE convoys behind the slow DVE
quanta, starving ACT. Per-k-tile DVE ops and wider pb buffering did not
recover it. All-ACT exp remains the best measured configuration.
"""

import math

import ml_dtypes
import numpy as np

D_MODEL = 128
N = 4096  # 64*64 positions
DH = 16  # head dim
NH = 8  # heads = cores
QC = 512  # q-chunk (one PSUM bank of fp32)
NQC = N // QC  # 8
KT = 128  # k positions per score tile
NKJ = N // KT  # 32
# k-tile groups: 3-wide (PE row groups 0/32/64) except the last
GROUPS = [(g * 3, min(3, NKJ - g * 3)) for g in range((NKJ + 2) // 3)]
SCALE = 1.0 / math.sqrt(D_MODEL)
# Schraudolph exp on DVE: i16 = floor(s_raw*EXP_C1 + EXP_C2) bitcast to bf16
# approximates exp(SCALE*s_raw); HW float->int convert truncates (verified).
# End-to-end rel err contribution ~1e-3 at this usage share (gate is 2e-2).
EXP_C1 = SCALE * 128.0 / math.log(2.0)
EXP_C2 = 127.0 * 128.0 - 7.5

_NC_CACHE = {}


def _build_nc(legalize=True, loop_reps=None, pb_bufs=3, pipelined=True):
    """Build the per-core Bass program. loop_reps wraps the whole body in a
    device-side For loop (used only for timing measurements)."""
    import concourse.bass as bass
    import concourse.mybir as mybir
    from concourse.tile import TileContext

    fp32 = mybir.dt.float32
    bf16 = mybir.dt.bfloat16
    EXP = mybir.ActivationFunctionType.Exp

    nc = bass.Bass(name="cmhs_attn_head")
    xb = nc.dram_tensor("xb", [D_MODEL, N], bf16, kind="ExternalInput")
    wq = nc.dram_tensor("wq", [D_MODEL, 128], bf16, kind="ExternalInput")
    wk = nc.dram_tensor("wk", [D_MODEL, 128], bf16, kind="ExternalInput")
    wv = nc.dram_tensor("wv", [D_MODEL, DH], bf16, kind="ExternalInput")
    out = nc.dram_tensor("out", [DH, N], fp32, kind="ExternalOutput")

    with (
        TileContext(nc) as tc,
        tc.tile_pool(name="const", bufs=1) as cpool,
        tc.tile_pool(name="pwork", bufs=pb_bufs) as ppool,
        tc.tile_pool(name="small", bufs=3) as mpool,
        tc.tile_pool(name="ps", bufs=2, space="PSUM") as pspool,
        tc.tile_pool(name="po", bufs=2, space="PSUM") as popool,
    ):
        if True:
            # ---- persistent SBUF tensors ----
            xb_sb = cpool.tile([D_MODEL, N], bf16, name="xb_sb")
            wq_sb = cpool.tile([D_MODEL, 128], bf16, name="wq_sb")
            wk_sb = cpool.tile([D_MODEL, 128], bf16, name="wk_sb")
            wv_sb = cpool.tile([D_MODEL, DH], bf16, name="wv_sb")
            qt = cpool.tile([D_MODEL, N], bf16, name="qt")  # replicated q^T
            kt = cpool.tile([D_MODEL, N], bf16, name="kt")  # replicated k^T
            # per k-chunk 48 cols: V (0:16) | zeros (16:32) | ones (32:48)
            v2 = cpool.tile([D_MODEL, NKJ * 48], bf16, name="v2")

            v2_v = v2.rearrange("p (j t) -> p j t", t=48)

            def proj_qk_group(dst, w_sb, c0, cn):
                # project q or k (replicated at partitions 0-15/32-47/64-79)
                # for x-chunks c0..c0+cn
                pj = pspool.tile([D_MODEL, 3 * QC], fp32, name="pj", tag="s")
                for t in range(cn):
                    c = c0 + t
                    nc.tensor.matmul(
                        pj[:, t * QC : (t + 1) * QC],
                        lhsT=w_sb[:],
                        rhs=xb_sb[:, c * QC : (c + 1) * QC],
                        start=True,
                        stop=True,
                    )
                # copy on ACT (Copy shares the Exp act-table: no reload);
                # DVE carries the trick-exp + normalize load instead
                nc.scalar.activation(
                    dst[:, c0 * QC : (c0 + cn) * QC],
                    pj[:, : cn * QC],
                    mybir.ActivationFunctionType.Copy,
                    scale=1.0,
                )

            def proj_v_range(vp, kj0, kj1):
                # V chunks kj0..kj1 position-major via x-chunk-stationary MMs
                vp_v = vp.rearrange("p (j t) -> p j t", t=DH)
                for kj in range(kj0, kj1):
                    nc.tensor.matmul(
                        vp[:, kj * DH : (kj + 1) * DH],
                        lhsT=xb_sb[:, kj * KT : (kj + 1) * KT],
                        rhs=wv_sb[:],
                        start=True,
                        stop=True,
                    )
                nc.vector.tensor_copy(
                    out=v2_v[:, kj0:kj1, 0:DH],
                    in_=vp_v[:, kj0:kj1, :],
                )

            def score_exp_group(qc, c0, cn, split):
                # packed score matmuls + exp over the group's PSUM span.
                # split=True (only for full groups): ACT exps k-tiles 0-1,
                # DVE bit-trick-exps k-tile 2. The DVE quantum is small
                # (512 free elems, ~1us incl pipe-drain) and the shared sps
                # tile frees at max(ACT, DVE) which are balanced, so the
                # 2-slot PSUM rotation never stalls the PE behind DVE.
                qs = qc * QC
                sps = pspool.tile([D_MODEL, 3 * QC], fp32, name="sps", tag="s")
                for t in range(cn):
                    kj = c0 + t
                    ro = 32 * t  # PE row group offset
                    nc.tensor.matmul(
                        sps[:, t * QC : (t + 1) * QC],
                        lhsT=kt[ro : ro + DH, kj * KT : (kj + 1) * KT],
                        rhs=qt[ro : ro + DH, qs : qs + QC],
                        start=True,
                        stop=True,
                    )
                pb = ppool.tile([D_MODEL, 3 * QC], bf16, name="pb", tag="p")
                n_act = 2 * QC if (split and cn == 3) else cn * QC
                nc.scalar.activation(
                    pb[:, :n_act], sps[:, :n_act], EXP, scale=SCALE
                )
                if n_act < cn * QC:
                    nc.vector.tensor_scalar(
                        out=pb[:, n_act : cn * QC].bitcast(mybir.dt.int16),
                        in0=sps[:, n_act : cn * QC],
                        scalar1=float(EXP_C1),
                        scalar2=float(EXP_C2),
                        op0=mybir.AluOpType.mult,
                        op1=mybir.AluOpType.add,
                    )
                return pb

            def ov_group(o2, pb, c0, cn, first, last):
                for t in range(cn):
                    kj = c0 + t
                    nc.tensor.matmul(
                        o2[:],
                        lhsT=v2[:, kj * 48 : kj * 48 + 48],
                        rhs=pb[:, t * QC : (t + 1) * QC],
                        start=(first and t == 0),
                        stop=(last and t == cn - 1),
                        skip_group_check=True,
                    )

            def normalize_and_store(qc, o2):
                # rows 32:48 of o2 all hold sum_k exp
                rcp = mpool.tile([DH, QC], fp32, name="rcp", tag="rcp")
                nc.vector.reciprocal(rcp[:], o2[32:48, :])
                ob = mpool.tile([DH, QC], fp32, name="ob", tag="ob")
                nc.vector.tensor_mul(ob[:], o2[0:DH, :], rcp[:])
                nc.sync.dma_start(out=out[:, qc * QC : (qc + 1) * QC], in_=ob[:])

            def body():
                # constant regions of v2 first: no data deps, runs at t=0
                nc.vector.memset(v2_v[:, :, DH:32], 0.0)
                nc.vector.memset(v2_v[:, :, 32:48], 1.0)

                nc.sync.dma_start(out=wq_sb[:], in_=wq[:])
                nc.sync.dma_start(out=wk_sb[:], in_=wk[:])
                nc.sync.dma_start(out=wv_sb[:], in_=wv[:])
                # x in halves so projection can start on the first half
                nc.sync.dma_start(out=xb_sb[:, : N // 2], in_=xb[:, : N // 2])
                nc.sync.dma_start(out=xb_sb[:, N // 2 :], in_=xb[:, N // 2 :])

                # Warm the ACT exp table (~2.7us load) immediately at t=0:
                # seed a tiny tile with DVE so the table DMA doesn't wait for
                # the weight DMA to land first.
                warm = mpool.tile([1, 32], bf16, name="warm", tag="warm")
                nc.vector.memset(warm[:], 0.25)
                nc.scalar.activation(warm[:], warm[:], EXP, scale=SCALE)

                # all projection up front; weaving it into the attention
                # measured consistently slower (it breaks the 2-slot PSUM
                # rotation that keeps the scalar engine saturated)
                for c0, cn in ((0, 3), (3, 3), (6, 2)):
                    proj_qk_group(qt, wq_sb, c0, cn)
                for c0, cn in ((0, 3), (3, 3), (6, 2)):
                    proj_qk_group(kt, wk_sb, c0, cn)
                vp = pspool.tile([D_MODEL, QC], fp32, name="vp", tag="s")
                proj_v_range(vp, 0, NKJ)

                # small group first: the first exp of each q-chunk fires
                # after only 2 score matmuls, ramping the scalar engine early
                groups = [GROUPS[-1]] + GROUPS[:-1]
                for qc in range(NQC):
                    o2 = popool.tile([48, QC], fp32, name="o2", tag="o")
                    # emit each group's score matmuls BEFORE the previous
                    # group's attention@V matmuls: the in-order PE then
                    # issues the score work the scalar engine needs next
                    # without stalling on exp(g)
                    pending = None
                    for gi, (c0, cn) in enumerate(groups):
                        # 9 of the 10 full groups split their last k-tile to
                        # DVE; gi==5 stays all-ACT to balance engine loads
                        pb = score_exp_group(qc, c0, cn, gi != 5)
                        if pending is not None:
                            ov_group(o2, *pending)
                        pending = (pb, c0, cn, gi == 0, gi == len(groups) - 1)
                    ov_group(o2, *pending)
                    normalize_and_store(qc, o2)

            if loop_reps is None:
                body()
            else:
                with tc.For_i(0, loop_reps, 1):
                    body()

    if legalize:
        # note: the inserted EventSemaphores are invisible to CoreSim's race
        # detector; build with legalize=False when simulating
        _legalize_pe_waits(nc)
    return nc


def _legalize_pe_waits(nc):
    """Several HW-decoded engine instruction formats (MM, AC, ...) have a
    single sync-wait slot, but Tile occasionally attaches 2-3 waits at
    slot-reuse boundaries. Hoist the extras onto EventSemaphore instructions
    (one wait each) on the same engine queue right before the instruction —
    the same mechanism as a standalone wait_ge."""
    import concourse.mybir as mybir

    skip = {"EventSemaphore", "Call"}
    n = 0
    for blk in nc.m.functions[0].blocks:
        insts = blk.instructions
        out = []
        changed = False
        for inst in insts:
            si = getattr(inst, "sync_info", None)
            if (
                inst.opcode not in skip
                and si is not None
                and si.on_wait
                and len(si.on_wait) > 1
            ):
                waits = list(si.on_wait)
                for w in waits[:-1]:
                    ev = mybir.InstEventSemaphore(
                        name=f"hoistwait_{inst.name}_{n}", ins=[], outs=[]
                    )
                    n += 1
                    ev.engine = inst.engine
                    ev.sync_info = mybir.SyncInfo(on_wait=[w], on_update=[])
                    out.append(ev)
                si.on_wait = [waits[-1]]
                changed = True
            out.append(inst)
        if changed:
            blk.instructions = out


def _get_nc():
    if "nc" not in _NC_CACHE:
        _NC_CACHE["nc"] = _build_nc()
    return _NC_CACHE["nc"]


def make_in_maps(x, W_qkv):
    """Host-side sharding: per-head input maps for the 8 cores."""
    bf16 = ml_dtypes.bfloat16
    x = np.asarray(x, dtype=np.float32).reshape(D_MODEL, N)
    W = np.asarray(W_qkv, dtype=np.float32)
    xb = np.ascontiguousarray(x.astype(bf16))
    in_maps = []
    for h in range(NH):
        Wq = W[48 * h : 48 * h + 16]
        Wk = W[48 * h + 16 : 48 * h + 32]
        Wv = W[48 * h + 32 : 48 * h + 48]
        wq_rep = np.zeros((D_MODEL, 128), dtype=bf16)
        wk_rep = np.zeros((D_MODEL, 128), dtype=bf16)
        for i in range(3):
            wq_rep[:, 32 * i : 32 * i + 16] = Wq.T.astype(bf16)
            wk_rep[:, 32 * i : 32 * i + 16] = Wk.T.astype(bf16)
        in_maps.append(
            {
                "xb": xb,
                "wq": wq_rep,
                "wk": wk_rep,
                "wv": np.ascontiguousarray(Wv.T.astype(bf16)),
            }
        )
    return in_maps


def run_spmd(x, W_qkv, **kwargs):
    """Compile + run on 8 cores; returns BassKernelResults."""
    from concourse.bass_utils import run_bass_kernel_spmd

    nc = _get_nc()
    in_maps = make_in_maps(x, W_qkv)
    return run_bass_kernel_spmd(nc, in_maps, core_ids=list(range(NH)), **kwargs)


def kernel(x, W_qkv):
    res = run_spmd(x, W_qkv)
    outs = [res.results[h]["out"] for h in range(NH)]  # each (16, 4096) fp32
    full = np.concatenate(outs, axis=0)  # (128, 4096)
    return np.ascontiguousarray(full.reshape(1, D_MODEL, 64, 64), dtype=np.float32)


# revision 24
# speedup vs baseline: 1.2755x; 1.2755x over previous
"""Multi-head self-attention (CMHSAttn) Trainium2 kernel.

Problem: x (1, 128, 64, 64) fp32, W_qkv (384, 128) fp32.
  qkv = 1x1-conv(x, W_qkv); per head h (8 heads, d_head=16):
  q,k,v from qkv channels [48h:48h+16], [48h+16:48h+32], [48h+32:48h+48];
  out = softmax(q k^T / sqrt(128)) v, laid out channel-major (128, 64, 64).

Sharding: one head per NeuronCore (8 cores), pure data parallel, no
collectives. Each core receives the full x (bf16, channel-major (128, 4096))
plus its head's weight slices, and computes its 16 output channels.

Per-core algorithm (all matmuls bf16 with fp32 PSUM accumulation):
  - QT/KT = W_{q,k} @ x, computed replicated at partition offsets 0/32/64
    so score matmuls can be packed 3-wide into PE row groups (K=16 only).
  - V2 (128, 48*32): per 128-position chunk kj, cols [48kj:48kj+16] = V chunk
    (position-major), [+16:+32] zeros, [+32:+48] = 1.0 (softmax-denominator
    rows; the zero padding keeps partition bases 32-aligned downstream).
  - For each q-chunk (512) and k-group (3 k-tiles of 128): S^T tiles
    (k-partition, q-free) via packed matmuls -> one ACT exp over (128, 1536)
    PSUM -> P bf16 -> matmuls accumulate O'' (48, 512) += V2_kj^T @ P_kj.
    O'' rows 0:16 = unnormalized output^T, rows 32:48 = softmax denominator.
  - out^T = O''[0:16] * reciprocal(O''[32:48]); DMA to HBM.

Optimization notes from the DVE-offload experiments (2026-08-08): splitting
the exp between ACT and a DVE Schraudolph bit-trick (int16(s*c1+c2) bitcast
to bf16; rel err ~6e-3 end-to-end, validated on HW) is numerically safe but
measured SLOWER (152-186us vs 147.7us): each DVE op pays a pipe-drain about
equal to its own duration, and the in-order PE convoys behind the slow DVE
quanta, starving ACT. Per-k-tile DVE ops and wider pb buffering did not
recover it. All-ACT exp remains the best measured configuration.
"""

import math

import ml_dtypes
import numpy as np

D_MODEL = 128
N = 4096  # 64*64 positions
DH = 16  # head dim
NH = 8  # heads = cores
QC = 512  # q-chunk (one PSUM bank of fp32)
NQC = N // QC  # 8
KT = 128  # k positions per score tile
NKJ = N // KT  # 32
# k-tile groups: 3-wide (PE row groups 0/32/64) except the last
GROUPS = [(g * 3, min(3, NKJ - g * 3)) for g in range((NKJ + 2) // 3)]
SCALE = 1.0 / math.sqrt(D_MODEL)

_NC_CACHE = {}


def _build_nc(legalize=True, loop_reps=None, pb_bufs=3, pipelined=True):
    """Build the per-core Bass program. loop_reps wraps the whole body in a
    device-side For loop (used only for timing measurements)."""
    import concourse.bass as bass
    import concourse.mybir as mybir
    from concourse.tile import TileContext

    fp32 = mybir.dt.float32
    bf16 = mybir.dt.bfloat16
    EXP = mybir.ActivationFunctionType.Exp

    nc = bass.Bass(name="cmhs_attn_head")
    xb = nc.dram_tensor("xb", [D_MODEL, N], bf16, kind="ExternalInput")
    wq = nc.dram_tensor("wq", [D_MODEL, 128], bf16, kind="ExternalInput")
    wk = nc.dram_tensor("wk", [D_MODEL, 128], bf16, kind="ExternalInput")
    wv = nc.dram_tensor("wv", [D_MODEL, DH], bf16, kind="ExternalInput")
    out = nc.dram_tensor("out", [DH, N], fp32, kind="ExternalOutput")

    with (
        TileContext(nc) as tc,
        tc.tile_pool(name="const", bufs=1) as cpool,
        tc.tile_pool(name="pwork", bufs=pb_bufs) as ppool,
        tc.tile_pool(name="small", bufs=3) as mpool,
        tc.tile_pool(name="ps", bufs=2, space="PSUM") as pspool,
        tc.tile_pool(name="po", bufs=2, space="PSUM") as popool,
    ):
        if True:
            # ---- persistent SBUF tensors ----
            xb_sb = cpool.tile([D_MODEL, N], bf16, name="xb_sb")
            wq_sb = cpool.tile([D_MODEL, 128], bf16, name="wq_sb")
            wk_sb = cpool.tile([D_MODEL, 128], bf16, name="wk_sb")
            wv_sb = cpool.tile([D_MODEL, DH], bf16, name="wv_sb")
            qt = cpool.tile([D_MODEL, N], bf16, name="qt")  # replicated q^T
            kt = cpool.tile([D_MODEL, N], bf16, name="kt")  # replicated k^T
            # per k-chunk 48 cols: V (0:16) | zeros (16:32) | ones (32:48)
            v2 = cpool.tile([D_MODEL, NKJ * 48], bf16, name="v2")

            v2_v = v2.rearrange("p (j t) -> p j t", t=48)

            def proj_qk_group(dst, w_sb, c0, cn):
                # project q or k (replicated at partitions 0-15/32-47/64-79)
                # for x-chunks c0..c0+cn
                pj = pspool.tile([D_MODEL, 3 * QC], fp32, name="pj", tag="s")
                for t in range(cn):
                    c = c0 + t
                    nc.tensor.matmul(
                        pj[:, t * QC : (t + 1) * QC],
                        lhsT=w_sb[:],
                        rhs=xb_sb[:, c * QC : (c + 1) * QC],
                        start=True,
                        stop=True,
                    )
                nc.vector.tensor_copy(
                    out=dst[:, c0 * QC : (c0 + cn) * QC],
                    in_=pj[:, : cn * QC],
                )

            def proj_v_range(vp, kj0, kj1):
                # V chunks kj0..kj1 position-major via x-chunk-stationary MMs
                vp_v = vp.rearrange("p (j t) -> p j t", t=DH)
                for kj in range(kj0, kj1):
                    nc.tensor.matmul(
                        vp[:, kj * DH : (kj + 1) * DH],
                        lhsT=xb_sb[:, kj * KT : (kj + 1) * KT],
                        rhs=wv_sb[:],
                        start=True,
                        stop=True,
                    )
                nc.vector.tensor_copy(
                    out=v2_v[:, kj0:kj1, 0:DH],
                    in_=vp_v[:, kj0:kj1, :],
                )

            def score_exp_group(qc, c0, cn):
                # packed score matmuls + one exp over the group's PSUM span
                qs = qc * QC
                sps = pspool.tile([D_MODEL, 3 * QC], fp32, name="sps", tag="s")
                for t in range(cn):
                    kj = c0 + t
                    ro = 32 * t  # PE row group offset
                    nc.tensor.matmul(
                        sps[:, t * QC : (t + 1) * QC],
                        lhsT=kt[ro : ro + DH, kj * KT : (kj + 1) * KT],
                        rhs=qt[ro : ro + DH, qs : qs + QC],
                        start=True,
                        stop=True,
                    )
                pb = ppool.tile([D_MODEL, 3 * QC], bf16, name="pb", tag="p")
                nc.scalar.activation(
                    pb[:, : cn * QC], sps[:, : cn * QC], EXP, scale=SCALE
                )
                return pb

            def ov_group(o2, pb, c0, cn, first, last):
                for t in range(cn):
                    kj = c0 + t
                    nc.tensor.matmul(
                        o2[:],
                        lhsT=v2[:, kj * 48 : kj * 48 + 48],
                        rhs=pb[:, t * QC : (t + 1) * QC],
                        start=(first and t == 0),
                        stop=(last and t == cn - 1),
                        skip_group_check=True,
                    )

            def normalize_and_store(qc, o2):
                # rows 32:48 of o2 all hold sum_k exp
                rcp = mpool.tile([DH, QC], fp32, name="rcp", tag="rcp")
                nc.vector.reciprocal(rcp[:], o2[32:48, :])
                ob = mpool.tile([DH, QC], fp32, name="ob", tag="ob")
                nc.vector.tensor_mul(ob[:], o2[0:DH, :], rcp[:])
                nc.sync.dma_start(out=out[:, qc * QC : (qc + 1) * QC], in_=ob[:])

            def body():
                # constant regions of v2 first: no data deps, runs at t=0
                nc.vector.memset(v2_v[:, :, DH:32], 0.0)
                nc.vector.memset(v2_v[:, :, 32:48], 1.0)

                nc.sync.dma_start(out=wq_sb[:], in_=wq[:])
                nc.sync.dma_start(out=wk_sb[:], in_=wk[:])
                nc.sync.dma_start(out=wv_sb[:], in_=wv[:])
                # x in halves so projection can start on the first half
                nc.sync.dma_start(out=xb_sb[:, : N // 2], in_=xb[:, : N // 2])
                nc.sync.dma_start(out=xb_sb[:, N // 2 :], in_=xb[:, N // 2 :])

                # Warm the ACT exp table (~2.7us load) immediately at t=0:
                # seed a tiny tile with DVE so the table DMA doesn't wait for
                # the weight DMA to land first.
                warm = mpool.tile([1, 32], bf16, name="warm", tag="warm")
                nc.vector.memset(warm[:], 0.25)
                nc.scalar.activation(warm[:], warm[:], EXP, scale=SCALE)

                # all projection up front; weaving it into the attention
                # measured consistently slower (it breaks the 2-slot PSUM
                # rotation that keeps the scalar engine saturated)
                for c0, cn in ((0, 3), (3, 3), (6, 2)):
                    proj_qk_group(qt, wq_sb, c0, cn)
                for c0, cn in ((0, 3), (3, 3), (6, 2)):
                    proj_qk_group(kt, wk_sb, c0, cn)
                vp = pspool.tile([D_MODEL, QC], fp32, name="vp", tag="s")
                proj_v_range(vp, 0, NKJ)

                # small group first: the first exp of each q-chunk fires
                # after only 2 score matmuls, ramping the scalar engine early
                groups = [GROUPS[-1]] + GROUPS[:-1]
                for qc in range(NQC):
                    o2 = popool.tile([48, QC], fp32, name="o2", tag="o")
                    # emit each group's score matmuls BEFORE the previous
                    # group's attention@V matmuls: the in-order PE then
                    # issues the score work the scalar engine needs next
                    # without stalling on exp(g)
                    pending = None
                    for gi, (c0, cn) in enumerate(groups):
                        pb = score_exp_group(qc, c0, cn)
                        if pending is not None:
                            ov_group(o2, *pending)
                        pending = (pb, c0, cn, gi == 0, gi == len(groups) - 1)
                    ov_group(o2, *pending)
                    normalize_and_store(qc, o2)

            if loop_reps is None:
                body()
            else:
                with tc.For_i(0, loop_reps, 1):
                    body()

    if legalize:
        # note: the inserted EventSemaphores are invisible to CoreSim's race
        # detector; build with legalize=False when simulating
        _legalize_pe_waits(nc)
    return nc


def _legalize_pe_waits(nc):
    """Several HW-decoded engine instruction formats (MM, AC, ...) have a
    single sync-wait slot, but Tile occasionally attaches 2-3 waits at
    slot-reuse boundaries. Hoist the extras onto EventSemaphore instructions
    (one wait each) on the same engine queue right before the instruction —
    the same mechanism as a standalone wait_ge."""
    import concourse.mybir as mybir

    skip = {"EventSemaphore", "Call"}
    n = 0
    for blk in nc.m.functions[0].blocks:
        insts = blk.instructions
        out = []
        changed = False
        for inst in insts:
            si = getattr(inst, "sync_info", None)
            if (
                inst.opcode not in skip
                and si is not None
                and si.on_wait
                and len(si.on_wait) > 1
            ):
                waits = list(si.on_wait)
                for w in waits[:-1]:
                    ev = mybir.InstEventSemaphore(
                        name=f"hoistwait_{inst.name}_{n}", ins=[], outs=[]
                    )
                    n += 1
                    ev.engine = inst.engine
                    ev.sync_info = mybir.SyncInfo(on_wait=[w], on_update=[])
                    out.append(ev)
                si.on_wait = [waits[-1]]
                changed = True
            out.append(inst)
        if changed:
            blk.instructions = out


def _get_nc():
    if "nc" not in _NC_CACHE:
        _NC_CACHE["nc"] = _build_nc()
    return _NC_CACHE["nc"]


def make_in_maps(x, W_qkv):
    """Host-side sharding: per-head input maps for the 8 cores."""
    bf16 = ml_dtypes.bfloat16
    x = np.asarray(x, dtype=np.float32).reshape(D_MODEL, N)
    W = np.asarray(W_qkv, dtype=np.float32)
    xb = np.ascontiguousarray(x.astype(bf16))
    in_maps = []
    for h in range(NH):
        Wq = W[48 * h : 48 * h + 16]
        Wk = W[48 * h + 16 : 48 * h + 32]
        Wv = W[48 * h + 32 : 48 * h + 48]
        wq_rep = np.zeros((D_MODEL, 128), dtype=bf16)
        wk_rep = np.zeros((D_MODEL, 128), dtype=bf16)
        for i in range(3):
            wq_rep[:, 32 * i : 32 * i + 16] = Wq.T.astype(bf16)
            wk_rep[:, 32 * i : 32 * i + 16] = Wk.T.astype(bf16)
        in_maps.append(
            {
                "xb": xb,
                "wq": wq_rep,
                "wk": wk_rep,
                "wv": np.ascontiguousarray(Wv.T.astype(bf16)),
            }
        )
    return in_maps


def run_spmd(x, W_qkv, **kwargs):
    """Compile + run on 8 cores; returns BassKernelResults."""
    from concourse.bass_utils import run_bass_kernel_spmd

    nc = _get_nc()
    in_maps = make_in_maps(x, W_qkv)
    return run_bass_kernel_spmd(nc, in_maps, core_ids=list(range(NH)), **kwargs)


def kernel(x, W_qkv):
    res = run_spmd(x, W_qkv)
    outs = [res.results[h]["out"] for h in range(NH)]  # each (16, 4096) fp32
    full = np.concatenate(outs, axis=0)  # (128, 4096)
    return np.ascontiguousarray(full.reshape(1, D_MODEL, 64, 64), dtype=np.float32)
